# revision 4
# baseline (speedup 1.0000x reference)
"""Trainium2 Bass kernel for nn_CrossModalFusion (single-head cross attention).

Per-batch-element cross attention, data-parallel over B=8 across 8 NeuronCores.

Per core (T=2048, D_RGB=400, D_POSE=256, H=512):
    q = rgb @ Wq + bq ; k = pose @ Wk + bk
    S = q @ k.T / sqrt(H) ; A = exp(S - ln 32)
    vp = pose @ (Wv @ Wp * FSCALE)        # Wv@Wp fused on host, fp8
    OT = vp.T @ A.T  (+ ones row 400 -> rowsums(A))
    y = rgb + bp + bv@Wp + (OT[:400].T / FSCALE) / OT[400]

Differences vs the previous kernel (v1, 110-130us):
  - Wv@Wp fused host-side: the y = (A@v)@Wp projection collapses into the
    O accumulation (O' = A @ vp directly in [dr, tq] orientation), removing
    32 y matmuls, 16 ot8 fp8 evictions and 16 y evictions, and the serial
    y tail.  O' evicts PSUM fp32 -> bf16 output directly (one quantization
    step instead of three).
  - rowsums(A) fold into O' via a ones column at dr=400 of vp (padding
    region): removes all 32 ones-vector sums matmuls + sums bank.
  - measured fp8 DoubleRow matmul throughput is 1 cycle/col (216ns per
    512-col tile) with LDWEIGHTS fully hidden, so the win is pure MM-count
    reduction: 390 -> ~334 matmuls.
"""

import sys

if "/opt/trn_rl_repo" not in sys.path:
    sys.path.insert(0, "/opt/trn_rl_repo")

from contextlib import ExitStack

import ml_dtypes
import numpy as np

import concourse.mybir as mybir
import concourse.tile as tile
from concourse import bacc, bass_utils

FP8 = mybir.dt.float8e4
F32 = mybir.dt.float32
BF16 = mybir.dt.bfloat16
NP_FP8 = ml_dtypes.float8_e4m3

B, T, DR, DP, H = 8, 2048, 400, 256, 512
PART = 128
DRP = 512                # rgb feature dim padded to 4*128
TQC = 512                # tq chunk width (max PSUM free dim)
NCH = T // TQC           # 4 chunks
NTK = T // PART          # 16 key tiles
NKP = NTK // 2           # 8 key tile pairs (DoubleRow)
NHT = H // PART          # 4 h tiles
NHP = NHT // 2           # 2 h tile pairs
NDR = DRP // PART        # 4 padded-rgb d tiles
NDRP = NDR // 2          # 2 pairs
NDP = DP // PART         # 2 pose d tiles
SCALE = float(1.0 / np.sqrt(np.float32(H)))
EXP_BIAS = float(-np.log(32.0))
FSCALE = 8.0             # host scales Wv@Wp by this into fp8 range

AT = mybir.ActivationFunctionType
OP = mybir.AluOpType
DRM = mybir.MatmulPerfMode.DoubleRow


def build_nc():
    nc = bacc.Bacc(
        "TRN2",
        target_bir_lowering=False,
        debug=False,
        enable_asserts=False,
        num_devices=8,
    )
    # all inputs arrive pre-permuted from the host: flat contiguous DMAs only
    # xq0 stays whole (early-critical); xq1-3 drop the 112 zero-padding
    # partitions of d-tile 3 (dr rows 400-511) to cut head-phase HBM traffic
    xq_d = [nc.dram_tensor("xq0", (PART, NDR, TQC), FP8, kind="ExternalInput").ap()]
    xqa_d = [
        nc.dram_tensor(f"xq{c}", (PART, 3, TQC), FP8, kind="ExternalInput").ap()
        for c in range(1, NCH)
    ]
    xqb_d = [
        nc.dram_tensor(f"xqb{c}", (32, 1, TQC), FP8, kind="ExternalInput").ap()
        for c in range(1, NCH)
    ]
    pq_d = [
        nc.dram_tensor(f"pq{c}", (PART, NDP, TQC), FP8, kind="ExternalInput").ap()
        for c in range(NCH)
    ]
    wka_d = nc.dram_tensor("wka", (PART, NDP, 2 * PART), FP8, kind="ExternalInput").ap()
    wkb_d = nc.dram_tensor("wkb", (PART, NDP, 2 * PART), FP8, kind="ExternalInput").ap()
    wf_d = nc.dram_tensor("wf", (PART, NDP, DRP), FP8, kind="ExternalInput").ap()
    wq_d = nc.dram_tensor("wq", (PART, NDR, H), FP8, kind="ExternalInput").ap()
    bqbk_d = nc.dram_tensor("bqbk", (PART, 2 * NHT), F32, kind="ExternalInput").ap()
    # yun4[c, p, d*TQC + t'] = y_unnorm[dr = d*128 + p, tq = c*TQC + t']
    # (chunk-major, full 128-partition shape: clean 4KB runs per partition;
    # rows 401-511 are padding garbage the host ignores, row 400 = rowsums)
    yun4 = nc.dram_tensor("yun4", (NCH, PART, NDR * TQC), BF16, kind="ExternalOutput").ap()

    with tile.TileContext(nc) as tc, ExitStack() as ctx:
        const = ctx.enter_context(tc.tile_pool(name="const", bufs=1))
        mm_ps = ctx.enter_context(tc.tile_pool(name="mm_ps", bufs=4, space="PSUM"))
        ot_ps = ctx.enter_context(tc.tile_pool(name="ot_ps", bufs=4, space="PSUM"))
        ex_pool = ctx.enter_context(tc.tile_pool(name="ex_pool", bufs=10))
        ysb_pool = ctx.enter_context(tc.tile_pool(name="ysb_pool", bufs=2))

        # ---- persistent inputs ----
        # flat contiguous DMAs, ordered per queue by first-use time
        wk8 = [const.tile([PART, NDP, 2 * PART], FP8, name=f"wk8_{i}") for i in range(2)]
        wf8 = const.tile([PART, NDP, DRP], FP8, name="wf8")
        wq8 = const.tile([PART, NDR, H], FP8, name="wq8")
        p8 = [const.tile([PART, NDP, TQC], FP8, name=f"p8_{c}") for c in range(NCH)]
        x8 = [const.tile([PART, NDR, TQC], FP8, name=f"x8_{c}") for c in range(NCH)]
        bqbk = const.tile([PART, 2 * NHT], F32, name="bqbk")

        # ones8 memset first: it gates the PE warm-up (the pad memsets
        # below aren't consumed until qT c1 mid-attention)
        ones8 = const.tile([PART, 2, TQC], FP8, name="ones8")
        nc.vector.memset(ones8[:], 1.0)
        # zero the d3 pad region of xq1-3 (the trimmed DMAs below only fill
        # partitions 0-31 of d-tile 3); emitted before the xqb DMAs so the
        # framework orders the overlapping writes after the memset
        for c in range(1, NCH):
            nc.vector.memset(x8[c][:, 3:4, :], 0.0)

        nc.scalar.dma_start(wk8[0][:], wka_d[:])
        nc.scalar.dma_start(wf8[:], wf_d[:])
        nc.scalar.dma_start(p8[1][:], pq_d[1][:])
        nc.scalar.dma_start(wq8[:], wq_d[:])
        nc.scalar.dma_start(x8[1][:, 0:3, :], xqa_d[0][:])
        nc.scalar.dma_start(x8[1][0:32, 3:4, :], xqb_d[0][:])

        nc.sync.dma_start(p8[0][:], pq_d[0][:])
        nc.sync.dma_start(p8[2][:], pq_d[2][:])
        nc.sync.dma_start(x8[0][:], xq_d[0][:])
        nc.sync.dma_start(x8[2][:, 0:3, :], xqa_d[1][:])
        nc.sync.dma_start(x8[2][0:32, 3:4, :], xqb_d[1][:])

        nc.gpsimd.dma_start(bqbk[:], bqbk_d[:])
        nc.gpsimd.dma_start(wk8[1][:], wkb_d[:])
        nc.gpsimd.dma_start(p8[3][:], pq_d[3][:])
        nc.gpsimd.dma_start(x8[3][:, 0:3, :], xqa_d[2][:])
        nc.gpsimd.dma_start(x8[3][0:32, 3:4, :], xqb_d[2][:])

        expb = const.tile([PART, 1], F32, name="expb")
        nc.vector.memset(expb[:], EXP_BIAS)

        # ---- persistent intermediates (fp8 DoubleRow pair layouts) ----
        # qT8[i2][p, s, t] = q[h = i2*256 + s*128 + p, t]
        qT8 = [const.tile([PART, 2, T], FP8, name=f"qT8_{i}") for i in range(NHP)]
        kT8 = [const.tile([PART, 2, T], FP8, name=f"kT8_{i}") for i in range(NHP)]
        # vp8[j2][p, s, d] = vp[tk = j2*256 + s*128 + p, d]; col 400 = 1.0
        vp8 = [const.tile([PART, 2, DRP], FP8, name=f"vp8_{j}") for j in range(NKP)]

        # ---- PE warm-up: burn the p-state ramp inside the DMA window ----
        wps = mm_ps.tile([PART, TQC], F32, name="warm", tag="mmps")
        NWARM = 9
        for w in range(NWARM):
            nc.tensor.matmul(
                wps[:], ones8[:, :, 0:PART], ones8[:],
                start=(w == 0), stop=(w == NWARM - 1),
                perf_mode=DRM,
            )
        warm_sink = const.tile([PART, PART], BF16, name="warm_sink")
        nc.vector.tensor_copy(warm_sink[:], wps[:, 0:PART])

        def evict_biased(n, dst, ps, bias_ap, scale):
            """PSUM->SBUF cast with scale*x+bias, alternating DVE/ACT."""
            if n % 2 == 0:
                if scale == 1.0:
                    nc.vector.tensor_scalar_add(dst, ps, bias_ap)
                else:
                    nc.vector.tensor_scalar(
                        dst, ps, scale, bias_ap, op0=OP.mult, op1=OP.add
                    )
            else:
                nc.scalar.activation(dst, ps, AT.Identity, bias=bias_ap, scale=scale)

        def emit_vp_tile(j, dve_evict):
            """vp[tk,dr'] = pT[dp,tk].T @ Wf[dp,dr'] -> fp8 for key tile j.
            Wf cols 400-511 are zero; after eviction col 400 is set to 1.0
            so the O' matmul's dr row 400 accumulates rowsums(A)."""
            ps = mm_ps.tile([PART, DRP], F32, name=f"vps_{j}", tag="mmps")
            nc.tensor.matmul(
                ps[:],
                p8[j // 4][:, :, (j % 4) * PART : (j % 4 + 1) * PART],
                wf8[:],
                start=True,
                stop=True,
                perf_mode=DRM,
            )
            if dve_evict:
                nc.vector.tensor_copy(vp8[j // 2][:, j % 2, :], ps[:])
            else:
                nc.scalar.copy(vp8[j // 2][:, j % 2, :], ps[:])
            nc.gpsimd.memset(vp8[j // 2][:, j % 2, DR : DR + 1], 1.0)

        def emit_qT_half(c, half, dve_evict=False):
            """qT[h,t] = Wq[d,h].T @ xT[d,t] + bq -> fp8, h tiles 2*half..2*half+1."""
            for i in (2 * half, 2 * half + 1):
                ps = mm_ps.tile([PART, TQC], F32, name=f"qps_{i}_{c}", tag="mmps")
                for d2 in range(NDRP):
                    nc.tensor.matmul(
                        ps[:],
                        wq8[:, 2 * d2 : 2 * d2 + 2, i * PART : (i + 1) * PART],
                        x8[c][:, 2 * d2 : 2 * d2 + 2, :],
                        start=(d2 == 0),
                        stop=(d2 == NDRP - 1),
                        perf_mode=DRM,
                    )
                dst = qT8[i // 2][:, i % 2, c * TQC : (c + 1) * TQC]
                if dve_evict:
                    nc.vector.tensor_scalar_add(dst, ps[:], bqbk[:, i : i + 1])
                else:
                    evict_biased(c * NHT + i + 1, dst, ps[:], bqbk[:, i : i + 1], 1.0)

        # ---- phase B: front-loaded projections ----
        # kT (all, scores need every key tile), vp tiles 0-7, qT chunk 0.
        for c in range(NCH):
            for i in range(NHT):
                ps = mm_ps.tile([PART, TQC], F32, name=f"kps_{i}_{c}", tag="mmps")
                nc.tensor.matmul(
                    ps[:],
                    wk8[i // 2][:, :, (i % 2) * PART : (i % 2 + 1) * PART],
                    p8[c][:],
                    start=True,
                    stop=True,
                    perf_mode=DRM,
                )
                evict_biased(
                    c * NHT + i,
                    kT8[i // 2][:, i % 2, c * TQC : (c + 1) * TQC],
                    ps[:],
                    bqbk[:, NHT + i : NHT + i + 1],
                    SCALE,
                )
            for jl in range(2):
                j = c * 2 + jl
                emit_vp_tile(j, dve_evict=(j % 2 == 0))
        emit_qT_half(0, 0)
        emit_qT_half(0, 1)

        # ---- phase C: attention, chunked over tq ----
        # Software pipeline: the O' group for score-pair p is emitted after
        # pair p+1's scores; the last pair's group lands after the next
        # chunk's first two score tiles.
        def emit_chunk_out(c, split):
            """Evict O' PSUM (fp32) -> bf16 ysb and DMA the chunk out.
            split=True (kernel end): ACT helps with casts and the DMA fans
            out across all five engine queues by partition range."""
            otps = chunk_state[c][0]
            ysb = ysb_pool.tile([PART, NDR, TQC], BF16, name=f"ysb_{c}", tag="ysb")
            if split:
                # exposed tail: pipeline per-tile evict -> DMA on alternating
                # queues so transfers start as soon as each tile is cast
                for d in range(NDR):
                    if d % 2 == 0:
                        nc.scalar.copy(ysb[:, d, :], otps[d][:])
                    else:
                        nc.vector.tensor_copy(ysb[:, d, :], otps[d][:])
                    q = (nc.sync, nc.gpsimd)[d % 2]
                    hi = 64 if d == NDR - 1 else PART
                    q.dma_start(
                        yun4[c, 0:hi, d * TQC : (d + 1) * TQC], ysb[0:hi, d, :]
                    )
            else:
                for d in range(NDR):
                    nc.vector.tensor_copy(ysb[:, d, :], otps[d][:])
                e = (nc.sync, nc.gpsimd)[c % 2]
                e.dma_start(yun4[c, :, :], ysb[:, :, :])

        chunk_state = {}  # c -> (otps, exs)

        def emit_scores_pair(c, p):
            exs = chunk_state[c][1]
            for j in (2 * p, 2 * p + 1):
                st = mm_ps.tile([PART, TQC], F32, name=f"st_{c}_{j}", tag="mmps")
                for i2 in range(NHP):
                    nc.tensor.matmul(
                        st[:],
                        kT8[i2][:, :, j * PART : (j + 1) * PART],
                        qT8[i2][:, :, c * TQC : (c + 1) * TQC],
                        start=(i2 == 0),
                        stop=(i2 == NHP - 1),
                        perf_mode=DRM,
                    )
                if j % 2 == 0:
                    ex = ex_pool.tile([PART, 2, TQC], FP8, name=f"ex_{c}_{j}", tag="ex")
                    exs.append(ex)
                nc.scalar.activation(exs[-1][:, j % 2, :], st[:], AT.Exp, bias=expb[:])

        def emit_o_group(c, j2):
            otps, exs = chunk_state[c]
            ex = exs[j2]
            for d in range(NDR):
                nc.tensor.matmul(
                    otps[d][:],
                    vp8[j2][:, :, d * PART : (d + 1) * PART],
                    ex[:],
                    start=(j2 == 0),
                    stop=(j2 == NKP - 1),
                    perf_mode=DRM,
                )

        for c in range(NCH):
            chunk_state[c] = (
                [
                    ot_ps.tile([PART, TQC], F32, name=f"otp_{c}_{d}", tag="otp")
                    for d in range(NDR)
                ],
                [],
            )
            for p in range(NKP):
                emit_scores_pair(c, p)
                if p == 1 and c > 0:
                    # previous chunk's deferred final O group + output
                    emit_o_group(c - 1, NKP - 1)
                    emit_chunk_out(c - 1, split=False)
                    del chunk_state[c - 1]
                if p > 0:
                    emit_o_group(c, p - 1)
                # deferred projections
                if c == 0:
                    if p in (1, 2, 3, 4):
                        emit_vp_tile(2 * p + 6, dve_evict=True)
                        emit_vp_tile(2 * p + 7, dve_evict=True)
                    elif p in (5, 6):
                        emit_qT_half(1, p - 5, dve_evict=True)
                else:
                    if c < NCH - 1 and p in (2, 3):
                        emit_qT_half(c + 1, p - 2, dve_evict=True)
        emit_o_group(NCH - 1, NKP - 1)
        emit_chunk_out(NCH - 1, split=True)

    nc.compile()
    return nc


_NC_CACHE = None


def get_nc():
    global _NC_CACHE
    if _NC_CACHE is None:
        _NC_CACHE = build_nc()
    return _NC_CACHE


def make_in_maps(rgb, pose, Wq, bq, Wk, bk, Wv, bv, Wp, bp):
    rgb = np.asarray(rgb, np.float32)
    pose = np.asarray(pose, np.float32)
    Wq, bq = np.asarray(Wq, np.float32), np.asarray(bq, np.float32)
    Wk, bk = np.asarray(Wk, np.float32), np.asarray(bk, np.float32)
    Wv = np.asarray(Wv, np.float32)
    Wp = np.asarray(Wp, np.float32)

    # xq[b][c][p, d, t] = rgb[b, c*512 + t, d*128 + p] (d padded 400->512)
    xT = np.zeros((B, DRP, T), NP_FP8)
    xT[:, :DR, :] = np.swapaxes(rgb, 1, 2).astype(NP_FP8)
    xP = xT.reshape(B, NDR, PART, NCH, TQC).transpose(0, 2, 1, 3, 4)  # b,p,d,c,t
    pT = np.swapaxes(pose, 1, 2).astype(NP_FP8)
    pP = pT.reshape(B, NDP, PART, NCH, TQC).transpose(0, 2, 1, 3, 4)

    wkp = Wk.astype(NP_FP8).reshape(NDP, PART, H).transpose(1, 0, 2)
    wq8f = np.zeros((DRP, H), np.float32)
    wq8f[:DR] = Wq
    wqp = wq8f.astype(NP_FP8).reshape(NDR, PART, H).transpose(1, 0, 2)
    # fused value->output weight, fp8, col 400-511 zero (400 becomes the
    # ones column on device for the rowsums fold)
    wff = np.zeros((DP, DRP), np.float32)
    wff[:, :DR] = (Wv @ Wp) * FSCALE
    wfp = wff.astype(NP_FP8).reshape(NDP, PART, DRP).transpose(1, 0, 2)
    bqbk = np.concatenate(
        [bq.reshape(NHT, PART).T, (bk * SCALE).reshape(NHT, PART).T], axis=1
    ).astype(np.float32)

    base = dict(
        wka=np.ascontiguousarray(wkp[:, :, : 2 * PART]),
        wkb=np.ascontiguousarray(wkp[:, :, 2 * PART :]),
        wf=np.ascontiguousarray(wfp),
        wq=np.ascontiguousarray(wqp),
        bqbk=np.ascontiguousarray(bqbk),
    )
    maps = []
    for b in range(B):
        m = dict(base)
        m["xq0"] = np.ascontiguousarray(xP[b, :, :, 0, :])
        for c in range(1, NCH):
            m[f"xq{c}"] = np.ascontiguousarray(xP[b, :, 0:3, c, :])
            m[f"xqb{c}"] = np.ascontiguousarray(xP[b, 0:32, 3:4, c, :])
        for c in range(NCH):
            m[f"pq{c}"] = np.ascontiguousarray(pP[b, :, :, c, :])
        maps.append(m)
    return maps


def kernel(rgb, pose, Wq, bq, Wk, bk, Wv, bv, Wp, bp):
    rgb = np.asarray(rgb, np.float32)
    Wp_f = np.asarray(Wp, np.float32)
    bp_eff = np.asarray(bp, np.float32) + np.asarray(bv, np.float32) @ Wp_f
    in_maps = make_in_maps(rgb, pose, Wq, bq, Wk, bk, Wv, bv, Wp, bp)
    res = bass_utils.run_bass_kernel_spmd(get_nc(), in_maps, core_ids=list(range(B)))
    out = np.empty((B, T, DR), np.float32)
    for b in range(B):
        yun4 = np.asarray(res.results[b]["yun4"]).astype(np.float32)
        # yun4[c, p, d*TQC+t'] -> full[d*128+p, c*TQC+t']
        full = yun4.reshape(NCH, PART, NDR, TQC).transpose(2, 1, 0, 3).reshape(DRP, T)
        yunT = full[:DR]
        sums = full[DR]  # [T]
        out[b] = rgb[b] + bp_eff + (yunT.T / FSCALE) / sums[:, None]
    return out
